# revision 31
# baseline (speedup 1.0000x reference)
"""Trainium2 Bass kernel for EquivariantPPFAttention (gnn_message_passing).

Contract: kernel(**inputs) takes FULL unsharded inputs (as produced by
reference.setup_inputs()) and returns the FULL [N, OUT, 3] float32 output.

Strategy (data-parallel over query points N across 8 NeuronCores):
  - shard q_pts / neighbor_indices across cores; replicate everything else.
  - one combined gather table comb[M, 512B]: s_feats row in bf16 (384B) +
    s_pts/normals in f32 (24B) + pad. dma_gather pulls 128*32 neighbor rows
    per query tile as 4 gathers of 1024 idxs, spread round-robin over 4
    SWDGE queues (descriptor generation runs on different Q7 core pairs
    concurrently -> ~2.7x faster than one queue).
  - fully pipelined per PAIR of query tiles: gather pair j+1 while pair j
    runs K-sum (bf16 tree adds on DVE), PPF geometry (DVE + ACT), the tiny
    MLP (TensorE, bf16), and the gated value path.
  - PPF angles: atan2(r,y) = atan(r/y) + pi*[y<0], with |a x b|^2 computed
    via the Lagrange identity |a|^2|b|^2 - (a.b)^2 (squared norms of the
    normals are precomputed on host into a spare comb slot). The 1/pi
    normalization folds into W1, mean-over-K into W3, b3 through Wg into
    bg, and 1/K of the value path into Wv.
  - two query-tiles packed per matmul via block-diagonal weights.
"""

import math
import numpy as np
import ml_dtypes

N = 20000
M = 20000
K = 32
D = 64
HID = 64
OUT = 192
PPF_OUT = 64
N_CORES = 8
PI = math.pi

ES = 128          # f32 elems per comb row (512 B)
SFW = 96          # f32 slots holding the 192 bf16 s_feats values
PNO = 96          # f32 slot offset of pts/normals/|normal|^2 (7 floats)
NI = 1024         # idxs per dma_gather (HW-stable limit)
GPT = (128 * K) // NI   # gathers per query tile (4)
KPG = K // GPT    # k-blocks per gather (8)
NQ_SW = 4         # SWDGE queues used round-robin

_NC_CACHE = {}


def _build_nc(T):
    """Per-core Bass program for T query-tiles of 128 (T even)."""
    from contextlib import ExitStack
    from concourse import bacc, bass, mybir, tile

    assert T % 2 == 0
    NPAIR = T // 2
    NQ = 128 * T
    f32 = mybir.dt.float32
    bf16 = mybir.dt.bfloat16
    i16 = mybir.dt.int16
    AF = mybir.ActivationFunctionType
    ALU = mybir.AluOpType

    nc = bacc.Bacc("TRN2", target_bir_lowering=False, debug=False,
                   num_swdge_queues=NQ_SW)

    comb_in = nc.dram_tensor("comb", [M, ES], f32, kind="ExternalInput")
    qp_in = nc.dram_tensor("qp", [128, T, 3], f32, kind="ExternalInput")
    idx_in = nc.dram_tensor("idx16", [128, T, GPT, NI // 16], i16,
                            kind="ExternalInput")
    w1b_in = nc.dram_tensor("w1b", [8, 128], bf16, kind="ExternalInput")
    b1b_in = nc.dram_tensor("b1b", [128, 1], f32, kind="ExternalInput")
    w2b_in = nc.dram_tensor("w2b", [128, 128], bf16, kind="ExternalInput")
    b2b_in = nc.dram_tensor("b2b", [128, 1], f32, kind="ExternalInput")
    w3b_in = nc.dram_tensor("w3b", [128, 128], f32, kind="ExternalInput")
    wgb_in = nc.dram_tensor("wgb", [128, 3, 128], f32, kind="ExternalInput")
    bgb_in = nc.dram_tensor("bgb", [128, 3], f32, kind="ExternalInput")
    wvb_in = nc.dram_tensor("wvb", [128, 3, 128], bf16, kind="ExternalInput")
    ident_in = nc.dram_tensor("ident", [128, 128], f32, kind="ExternalInput")
    out_dev = nc.dram_tensor("out", [3, OUT, NQ], f32, kind="ExternalOutput")

    with tile.TileContext(nc) as tc, ExitStack() as ctx:
        const = ctx.enter_context(tc.tile_pool(name="const", bufs=1))
        gpool = ctx.enter_context(tc.tile_pool(name="gpool", bufs=3))
        tpool = ctx.enter_context(tc.tile_pool(name="tpool", bufs=1))
        sfpool = ctx.enter_context(tc.tile_pool(name="sfpool", bufs=3))
        pnpool = ctx.enter_context(tc.tile_pool(name="pnpool", bufs=2))
        planes = ctx.enter_context(tc.tile_pool(name="planes", bufs=2))
        temps = ctx.enter_context(tc.tile_pool(name="temps", bufs=2))
        mlpp = ctx.enter_context(tc.tile_pool(name="mlpp", bufs=1))
        small = ctx.enter_context(tc.tile_pool(name="small", bufs=2))
        psmlp = ctx.enter_context(tc.tile_pool(name="psmlp", bufs=2, space="PSUM"))
        pssm = ctx.enter_context(tc.tile_pool(name="pssm", bufs=2, space="PSUM"))

        def cload(name, dram, shape, dt=f32):
            t = const.tile(shape, dt, tag=name, name=name)
            if len(shape) > 3:
                dims = " ".join(f"d{i}" for i in range(len(shape) - 1))
                pat = f"p {dims} -> p ({dims})"
                nc.sync.dma_start(t[:].rearrange(pat), dram.ap().rearrange(pat))
            else:
                nc.sync.dma_start(t[:], dram.ap())
            return t

        qp_t = cload("qp", qp_in, [128, T, 3])
        idx_t = cload("idx16", idx_in, [128, T, GPT, NI // 16], i16)
        w1b_t = cload("w1b", w1b_in, [8, 128], bf16)
        b1b_t = cload("b1b", b1b_in, [128, 1])
        w2b_t = cload("w2b", w2b_in, [128, 128], bf16)
        b2b_t = cload("b2b", b2b_in, [128, 1])
        w3b_t = cload("w3b", w3b_in, [128, 128])
        wgb_t = cload("wgb", wgb_in, [128, 3, 128])
        bgb_t = cload("bgb", bgb_in, [128, 3])
        wvb_t = cload("wvb", wvb_in, [128, 3, 128], bf16)
        ident_t = cload("ident", ident_in, [128, 128])
        zt = const.tile([128, 1], f32, tag="zt", name="zt")
        nc.vector.memset(zt[:], 0.0)

        out_re = out_dev.ap().rearrange("c (jj p) q -> p c jj q", jj=3)
        TT = nc.vector.tensor_tensor
        STT = nc.vector.scalar_tensor_tensor

        RW = 128 * K        # MLP rows per query tile (4096)
        HC = RW // 2        # rows per hh half (2048)
        def stage_front(j):
            # ---- gather the pair's 2*128*K neighbor rows ----
            gt = gpool.tile([128, 2, K, ES], f32, tag="gt", name="gt")
            for t2 in range(2):
                for g in range(GPT):
                    nc.gpsimd.dma_gather(
                        out_ap=gt[:, t2, g * KPG : (g + 1) * KPG, :],
                        in_ap=comb_in.ap(),
                        idxs_ap=idx_t[:, 2 * j + t2, g, :],
                        num_idxs=NI,
                        num_idxs_reg=NI,
                        elem_size=ES,
                        queue_num=(j * 2 * GPT + t2 * GPT + g) % NQ_SW,
                    )

            # ---- K-sum of bf16 s_feats: tree adds (contiguous reads) ----
            gtb = gt[:].bitcast(bf16)          # [128, 2, K, 256]
            s16 = tpool.tile([128, 2, 16, 192], bf16, tag="s16")
            TT(s16[:], gtb[:, :, 0:16, 0:192], gtb[:, :, 16:32, 0:192], ALU.add)
            s8 = tpool.tile([128, 2, 8, 192], bf16, tag="s8")
            TT(s8[:], s16[:, :, 0:8, :], s16[:, :, 8:16, :], ALU.add)
            s4 = tpool.tile([128, 2, 4, 192], bf16, tag="s4")
            TT(s4[:], s8[:, :, 0:4, :], s8[:, :, 4:8, :], ALU.add)
            s2 = tpool.tile([128, 2, 2, 192], bf16, tag="s2")
            TT(s2[:], s4[:, :, 0:2, :], s4[:, :, 2:4, :], ALU.add)
            sfs = sfpool.tile([128, 2, 192], f32, tag="sfs")
            TT(sfs[:], s2[:, :, 0, :], s2[:, :, 1, :], ALU.add)

            # ---- pack pts/normals/|n|^2 for the pair (ACT copy) ----
            pnb = pnpool.tile([128, 2, K, 8], f32, tag="pnb")
            nc.sync.dma_start(pnb[:, :, :, 0:7], gt[:, :, :, PNO : PNO + 7])

            # ---- PPF geometric features, stage-batched [128, 3|4, 2, K] ----
            def ttile(tag, shape=None):
                return temps.tile(shape or [128, 2, K], f32, tag=tag, name=tag)

            def np_c(c):
                return pnb[:, :, :, c]

            def nn_c(c):
                return pnb[:, :, :, 3 + c]

            def qn_c(c):
                return pnb[:, :, 0, 3 + c].to_broadcast([128, 2, K])

            def qp_c(c):
                return qp_t[:, 2 * j : 2 * j + 2, c].to_broadcast([128, 2, K])

            vd = []
            for c in range(3):
                t_ = ttile(f"vd{c}")
                TT(t_[:], np_c(c), qp_c(c), ALU.subtract)
                vd.append(t_)

            def dot_into(out_ap, av, bv, stag):
                m0 = temps.tile([128, 2, K], f32, tag="dm0", bufs=3)
                TT(m0[:], av[0], bv[0], ALU.mult)
                m1 = temps.tile([128, 2, K], f32, tag="dm1", bufs=3)
                TT(m1[:], av[1], bv[1], ALU.mult)
                TT(out_ap, m0[:], m1[:], ALU.add)
                m2 = temps.tile([128, 2, K], f32, tag="dm2", bufs=3)
                TT(m2[:], av[2], bv[2], ALU.mult)
                TT(out_ap, out_ap, m2[:], ALU.add)

            vdv = [t_[:] for t_ in vd]
            qnv = [qn_c(c) for c in range(3)]
            nnv = [nn_c(c) for c in range(3)]
            qn2 = pnb[:, :, 0, 6].to_broadcast([128, 2, K])
            nn2 = pnb[:, :, :, 6]

            # q4 slots: [dd, rs1, rs2, rs3]; y3t: the three dot products
            q4 = ttile("q4", [128, 4, 2, K])
            y3t = ttile("y3t", [128, 3, 2, K])
            dd = q4[:, 0]
            dot_into(dd, vdv, vdv, "sdd")
            for i, (av, bv) in enumerate(((qnv, vdv), (nnv, vdv), (qnv, nnv))):
                dot_into(y3t[:, i], av, bv, f"sy{i}")

            # |a x b|^2 = |a|^2 |b|^2 - (a.b)^2  (Lagrange), clamped at 0
            ysq = ttile("ysq", [128, 3, 2, K])
            TT(ysq[:], y3t[:], y3t[:], ALU.mult)
            TT(q4[:, 1], qn2, dd, ALU.mult)
            TT(q4[:, 2], nn2, dd, ALU.mult)
            TT(q4[:, 3], qn2, nn2, ALU.mult)
            TT(q4[:, 1:4], q4[:, 1:4], ysq[:], ALU.subtract)
            q4c = ttile("q4c", [128, 4, 2, K])
            TT(q4c[:], q4[:], zt[:, 0].to_broadcast([128, 4, 2, K]), ALU.max)
            rq4 = ttile("rq4", [128, 4, 2, K])
            nc.scalar.activation(rq4[:], q4c[:], AF.Sqrt)
            d_pl = planes.tile([128, 2, K], bf16, tag="d_pl", name="d_pl")
            nc.scalar.copy(d_pl[:], rq4[:, 0])

            ind3 = ttile("ind3", [128, 3, 2, K])
            TT(ind3[:], y3t[:], zt[:, 0].to_broadcast([128, 3, 2, K]), ALU.is_lt)
            iy3 = ttile("iy3", [128, 3, 2, K])
            nc.vector.reciprocal(iy3[:], y3t[:])
            tq3 = ttile("tq3", [128, 3, 2, K])
            TT(tq3[:], rq4[:, 1:4], iy3[:], ALU.mult)
            at3 = ttile("at3", [128, 3, 2, K])
            nc.scalar.activation(at3[:], tq3[:], AF.Arctan)
            apl = planes.tile([128, 3, 2, K], bf16, tag="apl", name="apl")
            STT(apl[:], ind3[:], PI, at3[:], ALU.mult, ALU.add)

            # ---- pack planes into MLP rows: pf[8, 4096] bf16 ----
            pf = mlpp.tile([8, RW], bf16, tag="pf", bufs=3)
            for t2 in range(2):
                nc.sync.dma_start(
                    pf[t2 * 4 : t2 * 4 + 1, :], d_pl[:, t2, :]
                )
                for ci in range(3):
                    nc.sync.dma_start(
                        pf[t2 * 4 + 1 + ci : t2 * 4 + 2 + ci, :],
                        apl[:, ci, t2, :],
                    )

            return sfs, pf

        def stage_back(j, sfs, pf):
            # ---- MLP (block-diagonal 2-tile packing) ----
            ksum = small.tile([128, 128], f32, tag="ksum")
            for hh in range(2):
                h1s = mlpp.tile([128, HC], bf16, tag="h1s", bufs=2)
                for ph in range(HC // 1024):
                    h1p = psmlp.tile([128, 1024], f32, tag="psmlp")
                    for ch in range(2):
                        slg = slice(hh * HC + ph * 1024 + ch * 512,
                                    hh * HC + ph * 1024 + (ch + 1) * 512)
                        nc.tensor.matmul(
                            h1p[:, ch * 512 : (ch + 1) * 512],
                            w1b_t[:], pf[:, slg], start=True, stop=True,
                        )
                    nc.scalar.activation(
                        h1s[:, ph * 1024 : (ph + 1) * 1024], h1p[:],
                        AF.Relu, bias=b1b_t[:],
                    )
                h2s = mlpp.tile([128, HC], bf16, tag="h2s", bufs=1)
                for ph in range(HC // 1024):
                    h2p = psmlp.tile([128, 1024], f32, tag="psmlp")
                    for ch in range(2):
                        sl = slice(ph * 1024 + ch * 512,
                                   ph * 1024 + (ch + 1) * 512)
                        nc.tensor.matmul(
                            h2p[:, ch * 512 : (ch + 1) * 512],
                            w2b_t[:], h1s[:, sl], start=True, stop=True,
                        )
                    nc.scalar.activation(
                        h2s[:, ph * 1024 : (ph + 1) * 1024], h2p[:],
                        AF.Relu, bias=b2b_t[:],
                    )
                nc.vector.reduce_sum(
                    ksum[:, hh * 64 : (hh + 1) * 64],
                    h2s[:].rearrange("p (q k) -> p q k", k=K),
                    mybir.AxisListType.X,
                )

            pmp = pssm.tile([128, 128], f32, tag="pssm")
            nc.tensor.matmul(pmp[:], w3b_t[:], ksum[:], start=True, stop=True)
            pms = small.tile([128, 128], f32, tag="pms", bufs=1)
            nc.scalar.copy(pms[:], pmp[:])  # b3 folded into bgb on host

            gates = []
            for jj in range(3):
                gp = pssm.tile([128, 128], f32, tag="pssm")
                nc.tensor.matmul(
                    gp[:], wgb_t[:, jj, :], pms[:], start=True, stop=True
                )
                gs = small.tile([128, 128], f32, tag=f"gate{jj}", name=f"gate{jj}")
                nc.scalar.activation(
                    gs[:], gp[:], AF.Sigmoid, bias=bgb_t[:, jj : jj + 1]
                )
                gates.append(gs)

            # ---- value path: transpose sfsum, then batched Wv matmuls ----
            av = sfs[:].rearrange("p t (d c) -> p c (t d)", c=3)
            aggs = small.tile([128, 3, 128], bf16, tag="aggs")
            for c in range(3):
                tp = pssm.tile([128, 128], f32, tag="pssm")
                nc.tensor.transpose(tp[:], av[:, c, :], ident_t[:])
                nc.scalar.copy(aggs[:, c, :], tp[:])
            vstage = small.tile([128, 3, 3, 128], f32, tag="vstage", bufs=1)
            for jj in range(3):
                vp = pssm.tile([128, 3, 128], f32, tag="psv")
                nc.tensor.matmul(
                    vp[:].rearrange("p c q -> p (c q)"),
                    wvb_t[:, jj, :],
                    aggs[:].rearrange("p c q -> p (c q)"),
                    start=True, stop=True,
                )
                for c in range(3):
                    TT(vstage[:, c, jj, :], vp[:, c, :], gates[jj][:], ALU.mult)

            for h in range(2):
                q0 = (2 * j + h) * 128
                nc.sync.dma_start(
                    out_re[:, :, :, q0 : q0 + 128].rearrange(
                        "p c jj q -> p (c jj) q"
                    ),
                    vstage[h * 64 : (h + 1) * 64, :, :, :].rearrange(
                        "p c jj q -> p (c jj) q"
                    ),
                )

        # software-pipelined issue: front of pair j alongside back of pair j-1
        staged = {}
        for j in range(NPAIR + 2):
            if j < NPAIR:
                staged[j] = stage_front(j)
            if j >= 2:
                stage_back(j - 2, *staged.pop(j - 2))

    nc.compile()
    return nc


def _f32_to_bf16_bits(x):
    """Round-to-nearest-even f32 -> bf16, returned as uint16 bits."""
    u = np.ascontiguousarray(x, dtype=np.float32).view(np.uint32)
    rounded = (u + 0x7FFF + ((u >> 16) & 1)) >> 16
    return rounded.astype(np.uint16)


def _host_prep(q_pts, s_pts, s_feats, neighbor_indices, normals,
               W1, b1, W2, b2, W3, b3, Wg, bg, Wv, T, n_total=N):
    NQ = 128 * T
    n_per_core = n_total // N_CORES
    f = np.float32
    bf = ml_dtypes.bfloat16

    comb = np.zeros((M, ES), dtype=f)
    cb = comb.view(np.uint16).reshape(M, ES * 2)
    cb[:, : 2 * SFW] = _f32_to_bf16_bits(s_feats.reshape(M, 192))
    comb[:, PNO : PNO + 3] = s_pts
    comb[:, PNO + 3 : PNO + 6] = normals
    comb[:, PNO + 6] = (normals.astype(f) ** 2).sum(axis=-1)

    W1T = W1.T.astype(f).copy()
    W1T[1:4] *= f(1.0 / PI)
    w1b = np.zeros((8, 128), dtype=f)
    w1b[0:4, 0:64] = W1T
    w1b[4:8, 64:128] = W1T
    b1b = np.concatenate([b1, b1]).astype(f)[:, None]

    def blockdiag2(A):
        n_, m_ = A.shape
        o = np.zeros((2 * n_, 2 * m_), dtype=f)
        o[:n_, :m_] = A
        o[n_:, m_:] = A
        return o

    w2b = blockdiag2(W2.T.astype(f))
    b2b = np.concatenate([b2, b2]).astype(f)[:, None]
    w3b = blockdiag2((W3.T / K).astype(f))
    gb3 = Wg.astype(f) @ b3.astype(f)  # b3 folded through the gate projection

    WgT = Wg.T.astype(f)
    WvT = (Wv.T / K).astype(f)
    wgb = np.zeros((3, 128, 128), dtype=f)
    wvb = np.zeros((3, 128, 128), dtype=f)
    bgb = np.zeros((128, 3), dtype=f)
    for jj in range(3):
        wgb[jj] = blockdiag2(WgT[:, jj * 64 : (jj + 1) * 64])
        wvb[jj] = blockdiag2(WvT[:, jj * 64 : (jj + 1) * 64])
        bgb[:, jj] = np.concatenate(
            [(bg + gb3)[jj * 64 : (jj + 1) * 64]] * 2
        )
    wgb_host = np.ascontiguousarray(wgb.transpose(1, 0, 2))
    wvb_host = np.ascontiguousarray(wvb.transpose(1, 0, 2)).astype(bf)
    ident = np.eye(128, dtype=f)

    shared = dict(
        comb=comb, w1b=w1b.astype(bf), b1b=b1b, w2b=w2b.astype(bf), b2b=b2b,
        w3b=w3b, wgb=wgb_host, bgb=bgb, wvb=wvb_host, ident=ident,
    )

    in_maps = []
    for i in range(N_CORES):
        lo = i * n_per_core
        hi = lo + n_per_core
        qp_pad = np.zeros((NQ, 3), dtype=f)
        qp_pad[: hi - lo] = q_pts[lo:hi]
        idx_pad = np.zeros((NQ, K), dtype=np.int64)
        idx_pad[: hi - lo] = neighbor_indices[lo:hi]

        qp_host = np.ascontiguousarray(qp_pad.reshape(T, 128, 3).transpose(1, 0, 2))

        # idx16[p, t, g, s]: gather g of tile t covers logical rows
        # i' = (k - g*KPG)*128 + q, wrapped: w[l, s] = list[s*16 + l]
        idx16 = np.zeros((128, T, GPT, NI // 16), np.int16)
        for t in range(T):
            arr = idx_pad[t * 128 : (t + 1) * 128, :]      # [128 q, K]
            for g in range(GPT):
                lst = arr[:, g * KPG : (g + 1) * KPG].T.reshape(NI)
                idx16[:, t, g, :] = np.tile(
                    lst.reshape(NI // 16, 16).T.astype(np.int16), (8, 1)
                )

        m = dict(shared)
        m.update(qp=qp_host, idx16=idx16)
        in_maps.append(m)
    return in_maps


def kernel(**inputs):
    from concourse.bass_utils import run_bass_kernel_spmd

    T = 20
    inputs = {k: np.asarray(v) for k, v in inputs.items()}
    idx = inputs["neighbor_indices"].astype(np.int64)

    if T not in _NC_CACHE:
        _NC_CACHE[T] = _build_nc(T)
    nc = _NC_CACHE[T]

    in_maps = _host_prep(
        inputs["q_pts"], inputs["s_pts"], inputs["s_feats"], idx,
        inputs["normals"], inputs["W1"], inputs["b1"], inputs["W2"],
        inputs["b2"], inputs["W3"], inputs["b3"], inputs["Wg"],
        inputs["bg"], inputs["Wv"], T,
    )
    res = run_bass_kernel_spmd(nc, in_maps, core_ids=list(range(N_CORES)))

    n_per_core = N // N_CORES
    out = np.empty((N, OUT, 3), dtype=np.float32)
    for i in range(N_CORES):
        o = np.asarray(res.results[i]["out"], dtype=np.float32)
        out[i * n_per_core : (i + 1) * n_per_core] = o.transpose(2, 1, 0)[:n_per_core]
    return out


# revision 32
# speedup vs baseline: 1.0023x; 1.0023x over previous
"""Trainium2 Bass kernel for EquivariantPPFAttention (gnn_message_passing).

Contract: kernel(**inputs) takes FULL unsharded inputs (as produced by
reference.setup_inputs()) and returns the FULL [N, OUT, 3] float32 output.

Strategy (data-parallel over query points N across 8 NeuronCores):
  - shard q_pts / neighbor_indices across cores; replicate everything else.
  - one combined gather table comb[M, 512B]: s_feats row in bf16 (384B) +
    s_pts/normals in f32 (24B) + pad. dma_gather pulls 128*32 neighbor rows
    per query tile as 4 gathers of 1024 idxs, spread round-robin over 4
    SWDGE queues (descriptor generation runs on different Q7 core pairs
    concurrently -> ~2.7x faster than one queue).
  - fully pipelined per PAIR of query tiles: gather pair j+1 while pair j
    runs K-sum (bf16 tree adds on DVE), PPF geometry (DVE + ACT), the tiny
    MLP (TensorE, bf16), and the gated value path.
  - PPF angles: atan2(r,y) = atan(r/y) + pi*[y<0], with |a x b|^2 computed
    via the Lagrange identity |a|^2|b|^2 - (a.b)^2 (squared norms of the
    normals are precomputed on host into a spare comb slot). The 1/pi
    normalization folds into W1, mean-over-K into W3, b3 through Wg into
    bg, and 1/K of the value path into Wv.
  - two query-tiles packed per matmul via block-diagonal weights.
"""

import math
import numpy as np
import ml_dtypes

N = 20000
M = 20000
K = 32
D = 64
HID = 64
OUT = 192
PPF_OUT = 64
N_CORES = 8
PI = math.pi

ES = 128          # f32 elems per comb row (512 B)
SFW = 96          # f32 slots holding the 192 bf16 s_feats values
PNO = 96          # f32 slot offset of pts/normals/|normal|^2 (7 floats)
NI = 1024         # idxs per dma_gather (HW-stable limit)
GPT = (128 * K) // NI   # gathers per query tile (4)
KPG = K // GPT    # k-blocks per gather (8)
NQ_SW = 4         # SWDGE queues used round-robin

_NC_CACHE = {}


def _build_nc(T):
    """Per-core Bass program for T query-tiles of 128 (T even)."""
    from contextlib import ExitStack
    from concourse import bacc, bass, mybir, tile

    assert T % 2 == 0
    NPAIR = T // 2
    NQ = 128 * T
    f32 = mybir.dt.float32
    bf16 = mybir.dt.bfloat16
    i16 = mybir.dt.int16
    AF = mybir.ActivationFunctionType
    ALU = mybir.AluOpType

    nc = bacc.Bacc("TRN2", target_bir_lowering=False, debug=False,
                   num_swdge_queues=NQ_SW)

    comb_in = nc.dram_tensor("comb", [M, ES], f32, kind="ExternalInput")
    qp_in = nc.dram_tensor("qp", [128, T, 3], f32, kind="ExternalInput")
    idx_in = nc.dram_tensor("idx16", [128, T, GPT, NI // 16], i16,
                            kind="ExternalInput")
    w1b_in = nc.dram_tensor("w1b", [8, 128], bf16, kind="ExternalInput")
    b1b_in = nc.dram_tensor("b1b", [128, 1], f32, kind="ExternalInput")
    w2b_in = nc.dram_tensor("w2b", [128, 128], bf16, kind="ExternalInput")
    b2b_in = nc.dram_tensor("b2b", [128, 1], f32, kind="ExternalInput")
    w3b_in = nc.dram_tensor("w3b", [128, 128], f32, kind="ExternalInput")
    wgb_in = nc.dram_tensor("wgb", [128, 3, 128], f32, kind="ExternalInput")
    bgb_in = nc.dram_tensor("bgb", [128, 3], f32, kind="ExternalInput")
    wvb_in = nc.dram_tensor("wvb", [128, 3, 128], bf16, kind="ExternalInput")
    ident_in = nc.dram_tensor("ident", [128, 128], f32, kind="ExternalInput")
    out_dev = nc.dram_tensor("out", [3, OUT, NQ], f32, kind="ExternalOutput")

    with tile.TileContext(nc) as tc, ExitStack() as ctx:
        const = ctx.enter_context(tc.tile_pool(name="const", bufs=1))
        gpool = ctx.enter_context(tc.tile_pool(name="gpool", bufs=3))
        tpool = ctx.enter_context(tc.tile_pool(name="tpool", bufs=1))
        sfpool = ctx.enter_context(tc.tile_pool(name="sfpool", bufs=3))
        pnpool = ctx.enter_context(tc.tile_pool(name="pnpool", bufs=2))
        planes = ctx.enter_context(tc.tile_pool(name="planes", bufs=2))
        temps = ctx.enter_context(tc.tile_pool(name="temps", bufs=2))
        mlpp = ctx.enter_context(tc.tile_pool(name="mlpp", bufs=1))
        small = ctx.enter_context(tc.tile_pool(name="small", bufs=2))
        psmlp = ctx.enter_context(tc.tile_pool(name="psmlp", bufs=2, space="PSUM"))
        pssm = ctx.enter_context(tc.tile_pool(name="pssm", bufs=2, space="PSUM"))

        def cload(name, dram, shape, dt=f32):
            t = const.tile(shape, dt, tag=name, name=name)
            if len(shape) > 3:
                dims = " ".join(f"d{i}" for i in range(len(shape) - 1))
                pat = f"p {dims} -> p ({dims})"
                nc.sync.dma_start(t[:].rearrange(pat), dram.ap().rearrange(pat))
            else:
                nc.sync.dma_start(t[:], dram.ap())
            return t

        qp_t = cload("qp", qp_in, [128, T, 3])
        idx_t = cload("idx16", idx_in, [128, T, GPT, NI // 16], i16)
        w1b_t = cload("w1b", w1b_in, [8, 128], bf16)
        b1b_t = cload("b1b", b1b_in, [128, 1])
        w2b_t = cload("w2b", w2b_in, [128, 128], bf16)
        b2b_t = cload("b2b", b2b_in, [128, 1])
        w3b_t = cload("w3b", w3b_in, [128, 128])
        wgb_t = cload("wgb", wgb_in, [128, 3, 128])
        bgb_t = cload("bgb", bgb_in, [128, 3])
        wvb_t = cload("wvb", wvb_in, [128, 3, 128], bf16)
        ident_t = cload("ident", ident_in, [128, 128])
        zt = const.tile([128, 1], f32, tag="zt", name="zt")
        nc.vector.memset(zt[:], 0.0)

        out_re = out_dev.ap().rearrange("c (jj p) q -> p c jj q", jj=3)
        TT = nc.vector.tensor_tensor
        STT = nc.vector.scalar_tensor_tensor

        RW = 128 * K        # MLP rows per query tile (4096)
        HC = RW // 2        # rows per hh half (2048)
        def stage_front(j):
            # ---- gather the pair's 2*128*K neighbor rows ----
            gt = gpool.tile([128, 2, K, ES], f32, tag="gt", name="gt")
            for t2 in range(2):
                for g in range(GPT):
                    nc.gpsimd.dma_gather(
                        out_ap=gt[:, t2, g * KPG : (g + 1) * KPG, :],
                        in_ap=comb_in.ap(),
                        idxs_ap=idx_t[:, 2 * j + t2, g, :],
                        num_idxs=NI,
                        num_idxs_reg=NI,
                        elem_size=ES,
                        queue_num=(j * 2 * GPT + t2 * GPT + g) % NQ_SW,
                    )

            # ---- K-sum of bf16 s_feats: tree adds (contiguous reads) ----
            gtb = gt[:].bitcast(bf16)          # [128, 2, K, 256]
            s16 = tpool.tile([128, 2, 16, 192], bf16, tag="s16")
            TT(s16[:], gtb[:, :, 0:16, 0:192], gtb[:, :, 16:32, 0:192], ALU.add)
            s8 = tpool.tile([128, 2, 8, 192], bf16, tag="s8")
            TT(s8[:], s16[:, :, 0:8, :], s16[:, :, 8:16, :], ALU.add)
            s4 = tpool.tile([128, 2, 4, 192], bf16, tag="s4")
            TT(s4[:], s8[:, :, 0:4, :], s8[:, :, 4:8, :], ALU.add)
            s2 = tpool.tile([128, 2, 2, 192], bf16, tag="s2")
            TT(s2[:], s4[:, :, 0:2, :], s4[:, :, 2:4, :], ALU.add)
            sfs = sfpool.tile([128, 2, 192], f32, tag="sfs")
            TT(sfs[:], s2[:, :, 0, :], s2[:, :, 1, :], ALU.add)

            # ---- pack pts/normals/|n|^2 for the pair (ACT copy) ----
            pnb = pnpool.tile([128, 2, K, 8], f32, tag="pnb")
            nc.sync.dma_start(pnb[:, :, :, 0:7], gt[:, :, :, PNO : PNO + 7])

            # ---- PPF geometric features, stage-batched [128, 3|4, 2, K] ----
            def ttile(tag, shape=None):
                return temps.tile(shape or [128, 2, K], f32, tag=tag, name=tag)

            def np_c(c):
                return pnb[:, :, :, c]

            def nn_c(c):
                return pnb[:, :, :, 3 + c]

            def qn_c(c):
                return pnb[:, :, 0, 3 + c].to_broadcast([128, 2, K])

            def qp_c(c):
                return qp_t[:, 2 * j : 2 * j + 2, c].to_broadcast([128, 2, K])

            vd = []
            for c in range(3):
                t_ = ttile(f"vd{c}")
                TT(t_[:], np_c(c), qp_c(c), ALU.subtract)
                vd.append(t_)

            def dot_into(out_ap, av, bv, stag):
                m0 = temps.tile([128, 2, K], f32, tag="dm0", bufs=3)
                TT(m0[:], av[0], bv[0], ALU.mult)
                m1 = temps.tile([128, 2, K], f32, tag="dm1", bufs=3)
                TT(m1[:], av[1], bv[1], ALU.mult)
                TT(out_ap, m0[:], m1[:], ALU.add)
                m2 = temps.tile([128, 2, K], f32, tag="dm2", bufs=3)
                TT(m2[:], av[2], bv[2], ALU.mult)
                TT(out_ap, out_ap, m2[:], ALU.add)

            vdv = [t_[:] for t_ in vd]
            qnv = [qn_c(c) for c in range(3)]
            nnv = [nn_c(c) for c in range(3)]
            qn2 = pnb[:, :, 0, 6].to_broadcast([128, 2, K])
            nn2 = pnb[:, :, :, 6]

            # q4 slots: [dd, rs1, rs2, rs3]; y3t: the three dot products
            q4 = ttile("q4", [128, 4, 2, K])
            y3t = ttile("y3t", [128, 3, 2, K])
            dd = q4[:, 0]
            dot_into(dd, vdv, vdv, "sdd")
            for i, (av, bv) in enumerate(((qnv, vdv), (nnv, vdv), (qnv, nnv))):
                dot_into(y3t[:, i], av, bv, f"sy{i}")

            # |a x b|^2 = |a|^2 |b|^2 - (a.b)^2  (Lagrange), clamped at 0
            ysq = ttile("ysq", [128, 3, 2, K])
            TT(ysq[:], y3t[:], y3t[:], ALU.mult)
            TT(q4[:, 1], qn2, dd, ALU.mult)
            TT(q4[:, 2], nn2, dd, ALU.mult)
            TT(q4[:, 3], qn2, nn2, ALU.mult)
            TT(q4[:, 1:4], q4[:, 1:4], ysq[:], ALU.subtract)
            q4c = ttile("q4c", [128, 4, 2, K])
            TT(q4c[:], q4[:], zt[:, 0].to_broadcast([128, 4, 2, K]), ALU.max)
            rq4 = ttile("rq4", [128, 4, 2, K])
            nc.scalar.activation(rq4[:], q4c[:], AF.Sqrt)
            d_pl = planes.tile([128, 2, K], bf16, tag="d_pl", name="d_pl")
            nc.vector.tensor_copy(d_pl[:], rq4[:, 0])

            ind3 = ttile("ind3", [128, 3, 2, K])
            TT(ind3[:], y3t[:], zt[:, 0].to_broadcast([128, 3, 2, K]), ALU.is_lt)
            iy3 = ttile("iy3", [128, 3, 2, K])
            nc.vector.reciprocal(iy3[:], y3t[:])
            tq3 = ttile("tq3", [128, 3, 2, K])
            TT(tq3[:], rq4[:, 1:4], iy3[:], ALU.mult)
            at3 = ttile("at3", [128, 3, 2, K])
            nc.scalar.activation(at3[:], tq3[:], AF.Arctan)
            apl = planes.tile([128, 3, 2, K], bf16, tag="apl", name="apl")
            STT(apl[:], ind3[:], PI, at3[:], ALU.mult, ALU.add)

            # ---- pack planes into MLP rows: pf[8, 4096] bf16 ----
            pf = mlpp.tile([8, RW], bf16, tag="pf", bufs=3)
            for t2 in range(2):
                nc.sync.dma_start(
                    pf[t2 * 4 : t2 * 4 + 1, :], d_pl[:, t2, :]
                )
                for ci in range(3):
                    nc.sync.dma_start(
                        pf[t2 * 4 + 1 + ci : t2 * 4 + 2 + ci, :],
                        apl[:, ci, t2, :],
                    )

            return sfs, pf

        def stage_back(j, sfs, pf):
            # ---- MLP (block-diagonal 2-tile packing) ----
            ksum = small.tile([128, 128], f32, tag="ksum")
            for hh in range(2):
                h1s = mlpp.tile([128, HC], bf16, tag="h1s", bufs=2)
                for ph in range(HC // 1024):
                    h1p = psmlp.tile([128, 1024], f32, tag="psmlp")
                    for ch in range(2):
                        slg = slice(hh * HC + ph * 1024 + ch * 512,
                                    hh * HC + ph * 1024 + (ch + 1) * 512)
                        nc.tensor.matmul(
                            h1p[:, ch * 512 : (ch + 1) * 512],
                            w1b_t[:], pf[:, slg], start=True, stop=True,
                        )
                    nc.scalar.activation(
                        h1s[:, ph * 1024 : (ph + 1) * 1024], h1p[:],
                        AF.Relu, bias=b1b_t[:],
                    )
                h2s = mlpp.tile([128, HC], bf16, tag="h2s", bufs=1)
                for ph in range(HC // 1024):
                    h2p = psmlp.tile([128, 1024], f32, tag="psmlp")
                    for ch in range(2):
                        sl = slice(ph * 1024 + ch * 512,
                                   ph * 1024 + (ch + 1) * 512)
                        nc.tensor.matmul(
                            h2p[:, ch * 512 : (ch + 1) * 512],
                            w2b_t[:], h1s[:, sl], start=True, stop=True,
                        )
                    nc.scalar.activation(
                        h2s[:, ph * 1024 : (ph + 1) * 1024], h2p[:],
                        AF.Relu, bias=b2b_t[:],
                    )
                nc.vector.reduce_sum(
                    ksum[:, hh * 64 : (hh + 1) * 64],
                    h2s[:].rearrange("p (q k) -> p q k", k=K),
                    mybir.AxisListType.X,
                )

            pmp = pssm.tile([128, 128], f32, tag="pssm")
            nc.tensor.matmul(pmp[:], w3b_t[:], ksum[:], start=True, stop=True)
            pms = small.tile([128, 128], f32, tag="pms", bufs=1)
            nc.scalar.copy(pms[:], pmp[:])  # b3 folded into bgb on host

            gates = []
            for jj in range(3):
                gp = pssm.tile([128, 128], f32, tag="pssm")
                nc.tensor.matmul(
                    gp[:], wgb_t[:, jj, :], pms[:], start=True, stop=True
                )
                gs = small.tile([128, 128], f32, tag=f"gate{jj}", name=f"gate{jj}")
                nc.scalar.activation(
                    gs[:], gp[:], AF.Sigmoid, bias=bgb_t[:, jj : jj + 1]
                )
                gates.append(gs)

            # ---- value path: transpose sfsum, then batched Wv matmuls ----
            av = sfs[:].rearrange("p t (d c) -> p c (t d)", c=3)
            aggs = small.tile([128, 3, 128], bf16, tag="aggs")
            for c in range(3):
                tp = pssm.tile([128, 128], f32, tag="pssm")
                nc.tensor.transpose(tp[:], av[:, c, :], ident_t[:])
                nc.scalar.copy(aggs[:, c, :], tp[:])
            vstage = small.tile([128, 3, 3, 128], f32, tag="vstage", bufs=1)
            for jj in range(3):
                vp = pssm.tile([128, 3, 128], f32, tag="psv")
                nc.tensor.matmul(
                    vp[:].rearrange("p c q -> p (c q)"),
                    wvb_t[:, jj, :],
                    aggs[:].rearrange("p c q -> p (c q)"),
                    start=True, stop=True,
                )
                for c in range(3):
                    TT(vstage[:, c, jj, :], vp[:, c, :], gates[jj][:], ALU.mult)

            for h in range(2):
                q0 = (2 * j + h) * 128
                nc.sync.dma_start(
                    out_re[:, :, :, q0 : q0 + 128].rearrange(
                        "p c jj q -> p (c jj) q"
                    ),
                    vstage[h * 64 : (h + 1) * 64, :, :, :].rearrange(
                        "p c jj q -> p (c jj) q"
                    ),
                )

        # software-pipelined issue: front of pair j alongside back of pair j-1
        staged = {}
        for j in range(NPAIR + 2):
            if j < NPAIR:
                staged[j] = stage_front(j)
            if j >= 2:
                stage_back(j - 2, *staged.pop(j - 2))

    nc.compile()
    return nc


def _f32_to_bf16_bits(x):
    """Round-to-nearest-even f32 -> bf16, returned as uint16 bits."""
    u = np.ascontiguousarray(x, dtype=np.float32).view(np.uint32)
    rounded = (u + 0x7FFF + ((u >> 16) & 1)) >> 16
    return rounded.astype(np.uint16)


def _host_prep(q_pts, s_pts, s_feats, neighbor_indices, normals,
               W1, b1, W2, b2, W3, b3, Wg, bg, Wv, T, n_total=N):
    NQ = 128 * T
    n_per_core = n_total // N_CORES
    f = np.float32
    bf = ml_dtypes.bfloat16

    comb = np.zeros((M, ES), dtype=f)
    cb = comb.view(np.uint16).reshape(M, ES * 2)
    cb[:, : 2 * SFW] = _f32_to_bf16_bits(s_feats.reshape(M, 192))
    comb[:, PNO : PNO + 3] = s_pts
    comb[:, PNO + 3 : PNO + 6] = normals
    comb[:, PNO + 6] = (normals.astype(f) ** 2).sum(axis=-1)

    W1T = W1.T.astype(f).copy()
    W1T[1:4] *= f(1.0 / PI)
    w1b = np.zeros((8, 128), dtype=f)
    w1b[0:4, 0:64] = W1T
    w1b[4:8, 64:128] = W1T
    b1b = np.concatenate([b1, b1]).astype(f)[:, None]

    def blockdiag2(A):
        n_, m_ = A.shape
        o = np.zeros((2 * n_, 2 * m_), dtype=f)
        o[:n_, :m_] = A
        o[n_:, m_:] = A
        return o

    w2b = blockdiag2(W2.T.astype(f))
    b2b = np.concatenate([b2, b2]).astype(f)[:, None]
    w3b = blockdiag2((W3.T / K).astype(f))
    gb3 = Wg.astype(f) @ b3.astype(f)  # b3 folded through the gate projection

    WgT = Wg.T.astype(f)
    WvT = (Wv.T / K).astype(f)
    wgb = np.zeros((3, 128, 128), dtype=f)
    wvb = np.zeros((3, 128, 128), dtype=f)
    bgb = np.zeros((128, 3), dtype=f)
    for jj in range(3):
        wgb[jj] = blockdiag2(WgT[:, jj * 64 : (jj + 1) * 64])
        wvb[jj] = blockdiag2(WvT[:, jj * 64 : (jj + 1) * 64])
        bgb[:, jj] = np.concatenate(
            [(bg + gb3)[jj * 64 : (jj + 1) * 64]] * 2
        )
    wgb_host = np.ascontiguousarray(wgb.transpose(1, 0, 2))
    wvb_host = np.ascontiguousarray(wvb.transpose(1, 0, 2)).astype(bf)
    ident = np.eye(128, dtype=f)

    shared = dict(
        comb=comb, w1b=w1b.astype(bf), b1b=b1b, w2b=w2b.astype(bf), b2b=b2b,
        w3b=w3b, wgb=wgb_host, bgb=bgb, wvb=wvb_host, ident=ident,
    )

    in_maps = []
    for i in range(N_CORES):
        lo = i * n_per_core
        hi = lo + n_per_core
        qp_pad = np.zeros((NQ, 3), dtype=f)
        qp_pad[: hi - lo] = q_pts[lo:hi]
        idx_pad = np.zeros((NQ, K), dtype=np.int64)
        idx_pad[: hi - lo] = neighbor_indices[lo:hi]

        qp_host = np.ascontiguousarray(qp_pad.reshape(T, 128, 3).transpose(1, 0, 2))

        # idx16[p, t, g, s]: gather g of tile t covers logical rows
        # i' = (k - g*KPG)*128 + q, wrapped: w[l, s] = list[s*16 + l]
        idx16 = np.zeros((128, T, GPT, NI // 16), np.int16)
        for t in range(T):
            arr = idx_pad[t * 128 : (t + 1) * 128, :]      # [128 q, K]
            for g in range(GPT):
                lst = arr[:, g * KPG : (g + 1) * KPG].T.reshape(NI)
                idx16[:, t, g, :] = np.tile(
                    lst.reshape(NI // 16, 16).T.astype(np.int16), (8, 1)
                )

        m = dict(shared)
        m.update(qp=qp_host, idx16=idx16)
        in_maps.append(m)
    return in_maps


def kernel(**inputs):
    from concourse.bass_utils import run_bass_kernel_spmd

    T = 20
    inputs = {k: np.asarray(v) for k, v in inputs.items()}
    idx = inputs["neighbor_indices"].astype(np.int64)

    if T not in _NC_CACHE:
        _NC_CACHE[T] = _build_nc(T)
    nc = _NC_CACHE[T]

    in_maps = _host_prep(
        inputs["q_pts"], inputs["s_pts"], inputs["s_feats"], idx,
        inputs["normals"], inputs["W1"], inputs["b1"], inputs["W2"],
        inputs["b2"], inputs["W3"], inputs["b3"], inputs["Wg"],
        inputs["bg"], inputs["Wv"], T,
    )
    res = run_bass_kernel_spmd(nc, in_maps, core_ids=list(range(N_CORES)))

    n_per_core = N // N_CORES
    out = np.empty((N, OUT, 3), dtype=np.float32)
    for i in range(N_CORES):
        o = np.asarray(res.results[i]["out"], dtype=np.float32)
        out[i * n_per_core : (i + 1) * n_per_core] = o.transpose(2, 1, 0)[:n_per_core]
    return out


# revision 33
# speedup vs baseline: 1.0084x; 1.0061x over previous
"""Trainium2 Bass kernel for EquivariantPPFAttention (gnn_message_passing).

Contract: kernel(**inputs) takes FULL unsharded inputs (as produced by
reference.setup_inputs()) and returns the FULL [N, OUT, 3] float32 output.

Strategy (data-parallel over query points N across 8 NeuronCores):
  - shard q_pts / neighbor_indices across cores; replicate everything else.
  - one combined gather table comb[M, 512B]: s_feats row in bf16 (384B) +
    s_pts/normals in f32 (24B) + pad. dma_gather pulls 128*32 neighbor rows
    per query tile as 4 gathers of 1024 idxs, spread round-robin over 4
    SWDGE queues (descriptor generation runs on different Q7 core pairs
    concurrently -> ~2.7x faster than one queue).
  - fully pipelined per PAIR of query tiles: gather pair j+1 while pair j
    runs K-sum (bf16 tree adds on DVE), PPF geometry (DVE + ACT), the tiny
    MLP (TensorE, bf16), and the gated value path.
  - PPF angles: atan2(r,y) = atan(r/y) + pi*[y<0], with |a x b|^2 computed
    via the Lagrange identity |a|^2|b|^2 - (a.b)^2 (squared norms of the
    normals are precomputed on host into a spare comb slot). The 1/pi
    normalization folds into W1, mean-over-K into W3, b3 through Wg into
    bg, and 1/K of the value path into Wv.
  - two query-tiles packed per matmul via block-diagonal weights.
"""

import math
import numpy as np
import ml_dtypes

N = 20000
M = 20000
K = 32
D = 64
HID = 64
OUT = 192
PPF_OUT = 64
N_CORES = 8
PI = math.pi

ES = 128          # f32 elems per comb row (512 B)
SFW = 96          # f32 slots holding the 192 bf16 s_feats values
PNO = 96          # f32 slot offset of pts/normals/|normal|^2 (7 floats)
NI = 1024         # idxs per dma_gather (HW-stable limit)
GPT = (128 * K) // NI   # gathers per query tile (4)
KPG = K // GPT    # k-blocks per gather (8)
NQ_SW = 4         # SWDGE queues used round-robin

_NC_CACHE = {}


def _build_nc(T):
    """Per-core Bass program for T query-tiles of 128 (T even)."""
    from contextlib import ExitStack
    from concourse import bacc, bass, mybir, tile

    assert T % 2 == 0
    NPAIR = T // 2
    NQ = 128 * T
    f32 = mybir.dt.float32
    bf16 = mybir.dt.bfloat16
    i16 = mybir.dt.int16
    AF = mybir.ActivationFunctionType
    ALU = mybir.AluOpType

    nc = bacc.Bacc("TRN2", target_bir_lowering=False, debug=False,
                   num_swdge_queues=NQ_SW)

    comb_in = nc.dram_tensor("comb", [M, ES], f32, kind="ExternalInput")
    qp_in = nc.dram_tensor("qp", [128, T, 3], f32, kind="ExternalInput")
    idx_in = nc.dram_tensor("idx16", [128, T, GPT, NI // 16], i16,
                            kind="ExternalInput")
    w1b_in = nc.dram_tensor("w1b", [8, 128], bf16, kind="ExternalInput")
    b1b_in = nc.dram_tensor("b1b", [128, 1], f32, kind="ExternalInput")
    w2b_in = nc.dram_tensor("w2b", [128, 128], bf16, kind="ExternalInput")
    b2b_in = nc.dram_tensor("b2b", [128, 1], f32, kind="ExternalInput")
    w3b_in = nc.dram_tensor("w3b", [128, 128], f32, kind="ExternalInput")
    wgb_in = nc.dram_tensor("wgb", [128, 3, 128], f32, kind="ExternalInput")
    bgb_in = nc.dram_tensor("bgb", [128, 3], f32, kind="ExternalInput")
    wvb_in = nc.dram_tensor("wvb", [128, 3, 128], bf16, kind="ExternalInput")
    ident_in = nc.dram_tensor("ident", [128, 128], f32, kind="ExternalInput")
    out_dev = nc.dram_tensor("out", [3, OUT, NQ], f32, kind="ExternalOutput")

    with tile.TileContext(nc) as tc, ExitStack() as ctx:
        const = ctx.enter_context(tc.tile_pool(name="const", bufs=1))
        gpool = ctx.enter_context(tc.tile_pool(name="gpool", bufs=3))
        tpool = ctx.enter_context(tc.tile_pool(name="tpool", bufs=1))
        sfpool = ctx.enter_context(tc.tile_pool(name="sfpool", bufs=3))
        pnpool = ctx.enter_context(tc.tile_pool(name="pnpool", bufs=2))
        planes = ctx.enter_context(tc.tile_pool(name="planes", bufs=2))
        temps = ctx.enter_context(tc.tile_pool(name="temps", bufs=2))
        mlpp = ctx.enter_context(tc.tile_pool(name="mlpp", bufs=1))
        small = ctx.enter_context(tc.tile_pool(name="small", bufs=2))
        psmlp = ctx.enter_context(tc.tile_pool(name="psmlp", bufs=2, space="PSUM"))
        pssm = ctx.enter_context(tc.tile_pool(name="pssm", bufs=2, space="PSUM"))

        def cload(name, dram, shape, dt=f32):
            t = const.tile(shape, dt, tag=name, name=name)
            if len(shape) > 3:
                dims = " ".join(f"d{i}" for i in range(len(shape) - 1))
                pat = f"p {dims} -> p ({dims})"
                nc.sync.dma_start(t[:].rearrange(pat), dram.ap().rearrange(pat))
            else:
                nc.sync.dma_start(t[:], dram.ap())
            return t

        qp_t = cload("qp", qp_in, [128, T, 3])
        idx_t = cload("idx16", idx_in, [128, T, GPT, NI // 16], i16)
        w1b_t = cload("w1b", w1b_in, [8, 128], bf16)
        b1b_t = cload("b1b", b1b_in, [128, 1])
        w2b_t = cload("w2b", w2b_in, [128, 128], bf16)
        b2b_t = cload("b2b", b2b_in, [128, 1])
        w3b_t = cload("w3b", w3b_in, [128, 128])
        wgb_t = cload("wgb", wgb_in, [128, 3, 128])
        bgb_t = cload("bgb", bgb_in, [128, 3])
        wvb_t = cload("wvb", wvb_in, [128, 3, 128], bf16)
        ident_t = cload("ident", ident_in, [128, 128])
        zt = const.tile([128, 1], f32, tag="zt", name="zt")
        nc.vector.memset(zt[:], 0.0)

        out_re = out_dev.ap().rearrange("c (jj p) q -> p c jj q", jj=3)
        TT = nc.vector.tensor_tensor
        STT = nc.vector.scalar_tensor_tensor

        RW = 128 * K        # MLP rows per query tile (4096)
        HC = RW // 2        # rows per hh half (2048)
        def stage_front(j):
            # ---- gather the pair's 2*128*K neighbor rows ----
            gt = gpool.tile([128, 2, K, ES], f32, tag="gt", name="gt")
            for t2 in range(2):
                for g in range(GPT):
                    nc.gpsimd.dma_gather(
                        out_ap=gt[:, t2, g * KPG : (g + 1) * KPG, :],
                        in_ap=comb_in.ap(),
                        idxs_ap=idx_t[:, 2 * j + t2, g, :],
                        num_idxs=NI,
                        num_idxs_reg=NI,
                        elem_size=ES,
                        queue_num=(j * 2 * GPT + t2 * GPT + g) % NQ_SW,
                    )

            # ---- K-sum of bf16 s_feats: tree adds (contiguous reads) ----
            gtb = gt[:].bitcast(bf16)          # [128, 2, K, 256]
            s16 = tpool.tile([128, 2, 16, 192], bf16, tag="s16")
            TT(s16[:], gtb[:, :, 0:16, 0:192], gtb[:, :, 16:32, 0:192], ALU.add)
            s8 = tpool.tile([128, 2, 8, 192], bf16, tag="s8")
            TT(s8[:], s16[:, :, 0:8, :], s16[:, :, 8:16, :], ALU.add)
            s4 = tpool.tile([128, 2, 4, 192], bf16, tag="s4")
            TT(s4[:], s8[:, :, 0:4, :], s8[:, :, 4:8, :], ALU.add)
            s2 = tpool.tile([128, 2, 2, 192], bf16, tag="s2")
            TT(s2[:], s4[:, :, 0:2, :], s4[:, :, 2:4, :], ALU.add)
            sfs = sfpool.tile([128, 2, 192], f32, tag="sfs")
            TT(sfs[:], s2[:, :, 0, :], s2[:, :, 1, :], ALU.add)

            # ---- pack pts/normals/|n|^2 for the pair (ACT copy) ----
            pnb = pnpool.tile([128, 2, K, 8], f32, tag="pnb")
            nc.sync.dma_start(pnb[:, :, :, 0:7], gt[:, :, :, PNO : PNO + 7])

            # ---- PPF geometric features, stage-batched [128, 3|4, 2, K] ----
            def ttile(tag, shape=None):
                return temps.tile(shape or [128, 2, K], f32, tag=tag, name=tag)

            def np_c(c):
                return pnb[:, :, :, c]

            def nn_c(c):
                return pnb[:, :, :, 3 + c]

            def qn_c(c):
                return pnb[:, :, 0, 3 + c].to_broadcast([128, 2, K])

            def qp_c(c):
                return qp_t[:, 2 * j : 2 * j + 2, c].to_broadcast([128, 2, K])

            vd = []
            for c in range(3):
                t_ = ttile(f"vd{c}")
                TT(t_[:], np_c(c), qp_c(c), ALU.subtract)
                vd.append(t_)

            def dot_into(out_ap, av, bv, stag):
                m0 = temps.tile([128, 2, K], f32, tag="dm0", bufs=3)
                TT(m0[:], av[0], bv[0], ALU.mult)
                m1 = temps.tile([128, 2, K], f32, tag="dm1", bufs=3)
                TT(m1[:], av[1], bv[1], ALU.mult)
                TT(out_ap, m0[:], m1[:], ALU.add)
                m2 = temps.tile([128, 2, K], f32, tag="dm2", bufs=3)
                TT(m2[:], av[2], bv[2], ALU.mult)
                TT(out_ap, out_ap, m2[:], ALU.add)

            vdv = [t_[:] for t_ in vd]
            qnv = [qn_c(c) for c in range(3)]
            nnv = [nn_c(c) for c in range(3)]
            qn2 = pnb[:, :, 0, 6].to_broadcast([128, 2, K])
            nn2 = pnb[:, :, :, 6]

            # q4 slots: [dd, rs1, rs2, rs3]; y3t: the three dot products
            q4 = ttile("q4", [128, 4, 2, K])
            y3t = ttile("y3t", [128, 3, 2, K])
            dd = q4[:, 0]
            dot_into(dd, vdv, vdv, "sdd")
            for i, (av, bv) in enumerate(((qnv, vdv), (nnv, vdv), (qnv, nnv))):
                dot_into(y3t[:, i], av, bv, f"sy{i}")

            # |a x b|^2 = |a|^2 |b|^2 - (a.b)^2  (Lagrange), clamped at 0
            ysq = ttile("ysq", [128, 3, 2, K])
            TT(ysq[:], y3t[:], y3t[:], ALU.mult)
            TT(q4[:, 1], qn2, dd, ALU.mult)
            TT(q4[:, 2], nn2, dd, ALU.mult)
            TT(q4[:, 3], qn2, nn2, ALU.mult)
            TT(q4[:, 1:4], q4[:, 1:4], ysq[:], ALU.subtract)
            q4c = ttile("q4c", [128, 4, 2, K])
            TT(q4c[:], q4[:], zt[:, 0].to_broadcast([128, 4, 2, K]), ALU.max)
            rq4 = ttile("rq4", [128, 4, 2, K])
            nc.scalar.activation(rq4[:], q4c[:], AF.Sqrt)
            d_pl = planes.tile([128, 2, K], bf16, tag="d_pl", name="d_pl")
            nc.vector.tensor_copy(d_pl[:], rq4[:, 0])

            ind3 = ttile("ind3", [128, 3, 2, K])
            TT(ind3[:], y3t[:], zt[:, 0].to_broadcast([128, 3, 2, K]), ALU.is_lt)
            iy3 = ttile("iy3", [128, 3, 2, K])
            nc.vector.reciprocal(iy3[:], y3t[:])
            tq3 = ttile("tq3", [128, 3, 2, K])
            TT(tq3[:], rq4[:, 1:4], iy3[:], ALU.mult)
            at3 = ttile("at3", [128, 3, 2, K])
            nc.scalar.activation(at3[:], tq3[:], AF.Arctan)
            apl = planes.tile([128, 3, 2, K], bf16, tag="apl", name="apl")
            STT(apl[:], ind3[:], PI, at3[:], ALU.mult, ALU.add)

            # ---- pack planes into MLP rows: pf[8, 4096] bf16 ----
            pf = mlpp.tile([8, RW], bf16, tag="pf", bufs=3)
            for t2 in range(2):
                nc.sync.dma_start(
                    pf[t2 * 4 : t2 * 4 + 1, :], d_pl[:, t2, :]
                )
                for ci in range(3):
                    nc.sync.dma_start(
                        pf[t2 * 4 + 1 + ci : t2 * 4 + 2 + ci, :],
                        apl[:, ci, t2, :],
                    )

            return sfs, pf

        def stage_back(j, sfs, pf):
            # ---- MLP (block-diagonal 2-tile packing) ----
            ksum = small.tile([128, 128], f32, tag="ksum")
            for hh in range(2):
                h1s = mlpp.tile([128, HC], bf16, tag="h1s", bufs=2)
                for ph in range(HC // 1024):
                    h1p = psmlp.tile([128, 1024], f32, tag="psmlp")
                    for ch in range(2):
                        slg = slice(hh * HC + ph * 1024 + ch * 512,
                                    hh * HC + ph * 1024 + (ch + 1) * 512)
                        nc.tensor.matmul(
                            h1p[:, ch * 512 : (ch + 1) * 512],
                            w1b_t[:], pf[:, slg], start=True, stop=True,
                        )
                    nc.scalar.activation(
                        h1s[:, ph * 1024 : (ph + 1) * 1024], h1p[:],
                        AF.Relu, bias=b1b_t[:],
                    )
                h2s = mlpp.tile([128, HC], bf16, tag="h2s", bufs=1)
                for ph in range(HC // 1024):
                    h2p = psmlp.tile([128, 1024], f32, tag="psmlp")
                    for ch in range(2):
                        sl = slice(ph * 1024 + ch * 512,
                                   ph * 1024 + (ch + 1) * 512)
                        nc.tensor.matmul(
                            h2p[:, ch * 512 : (ch + 1) * 512],
                            w2b_t[:], h1s[:, sl], start=True, stop=True,
                        )
                    nc.scalar.activation(
                        h2s[:, ph * 1024 : (ph + 1) * 1024], h2p[:],
                        AF.Relu, bias=b2b_t[:],
                    )
                nc.vector.reduce_sum(
                    ksum[:, hh * 64 : (hh + 1) * 64],
                    h2s[:].rearrange("p (q k) -> p q k", k=K),
                    mybir.AxisListType.X,
                )

            pmp = pssm.tile([128, 128], f32, tag="pssm")
            nc.tensor.matmul(pmp[:], w3b_t[:], ksum[:], start=True, stop=True)
            pms = small.tile([128, 128], f32, tag="pms", bufs=1)
            nc.vector.tensor_copy(pms[:], pmp[:])  # b3 folded into bgb on host

            gates = []
            for jj in range(3):
                gp = pssm.tile([128, 128], f32, tag="pssm")
                nc.tensor.matmul(
                    gp[:], wgb_t[:, jj, :], pms[:], start=True, stop=True
                )
                gs = small.tile([128, 128], f32, tag=f"gate{jj}", name=f"gate{jj}")
                nc.scalar.activation(
                    gs[:], gp[:], AF.Sigmoid, bias=bgb_t[:, jj : jj + 1]
                )
                gates.append(gs)

            # ---- value path: transpose sfsum, then batched Wv matmuls ----
            av = sfs[:].rearrange("p t (d c) -> p c (t d)", c=3)
            aggs = small.tile([128, 3, 128], bf16, tag="aggs")
            for c in range(3):
                tp = pssm.tile([128, 128], f32, tag="pssm")
                nc.tensor.transpose(tp[:], av[:, c, :], ident_t[:])
                nc.vector.tensor_copy(aggs[:, c, :], tp[:])
            vstage = small.tile([128, 3, 3, 128], f32, tag="vstage", bufs=1)
            for jj in range(3):
                vp = pssm.tile([128, 3, 128], f32, tag="psv")
                nc.tensor.matmul(
                    vp[:].rearrange("p c q -> p (c q)"),
                    wvb_t[:, jj, :],
                    aggs[:].rearrange("p c q -> p (c q)"),
                    start=True, stop=True,
                )
                for c in range(3):
                    TT(vstage[:, c, jj, :], vp[:, c, :], gates[jj][:], ALU.mult)

            for h in range(2):
                q0 = (2 * j + h) * 128
                nc.sync.dma_start(
                    out_re[:, :, :, q0 : q0 + 128].rearrange(
                        "p c jj q -> p (c jj) q"
                    ),
                    vstage[h * 64 : (h + 1) * 64, :, :, :].rearrange(
                        "p c jj q -> p (c jj) q"
                    ),
                )

        # software-pipelined issue: front of pair j alongside back of pair j-1
        staged = {}
        for j in range(NPAIR + 2):
            if j < NPAIR:
                staged[j] = stage_front(j)
            if j >= 2:
                stage_back(j - 2, *staged.pop(j - 2))

    nc.compile()
    return nc


def _f32_to_bf16_bits(x):
    """Round-to-nearest-even f32 -> bf16, returned as uint16 bits."""
    u = np.ascontiguousarray(x, dtype=np.float32).view(np.uint32)
    rounded = (u + 0x7FFF + ((u >> 16) & 1)) >> 16
    return rounded.astype(np.uint16)


def _host_prep(q_pts, s_pts, s_feats, neighbor_indices, normals,
               W1, b1, W2, b2, W3, b3, Wg, bg, Wv, T, n_total=N):
    NQ = 128 * T
    n_per_core = n_total // N_CORES
    f = np.float32
    bf = ml_dtypes.bfloat16

    comb = np.zeros((M, ES), dtype=f)
    cb = comb.view(np.uint16).reshape(M, ES * 2)
    cb[:, : 2 * SFW] = _f32_to_bf16_bits(s_feats.reshape(M, 192))
    comb[:, PNO : PNO + 3] = s_pts
    comb[:, PNO + 3 : PNO + 6] = normals
    comb[:, PNO + 6] = (normals.astype(f) ** 2).sum(axis=-1)

    W1T = W1.T.astype(f).copy()
    W1T[1:4] *= f(1.0 / PI)
    w1b = np.zeros((8, 128), dtype=f)
    w1b[0:4, 0:64] = W1T
    w1b[4:8, 64:128] = W1T
    b1b = np.concatenate([b1, b1]).astype(f)[:, None]

    def blockdiag2(A):
        n_, m_ = A.shape
        o = np.zeros((2 * n_, 2 * m_), dtype=f)
        o[:n_, :m_] = A
        o[n_:, m_:] = A
        return o

    w2b = blockdiag2(W2.T.astype(f))
    b2b = np.concatenate([b2, b2]).astype(f)[:, None]
    w3b = blockdiag2((W3.T / K).astype(f))
    gb3 = Wg.astype(f) @ b3.astype(f)  # b3 folded through the gate projection

    WgT = Wg.T.astype(f)
    WvT = (Wv.T / K).astype(f)
    wgb = np.zeros((3, 128, 128), dtype=f)
    wvb = np.zeros((3, 128, 128), dtype=f)
    bgb = np.zeros((128, 3), dtype=f)
    for jj in range(3):
        wgb[jj] = blockdiag2(WgT[:, jj * 64 : (jj + 1) * 64])
        wvb[jj] = blockdiag2(WvT[:, jj * 64 : (jj + 1) * 64])
        bgb[:, jj] = np.concatenate(
            [(bg + gb3)[jj * 64 : (jj + 1) * 64]] * 2
        )
    wgb_host = np.ascontiguousarray(wgb.transpose(1, 0, 2))
    wvb_host = np.ascontiguousarray(wvb.transpose(1, 0, 2)).astype(bf)
    ident = np.eye(128, dtype=f)

    shared = dict(
        comb=comb, w1b=w1b.astype(bf), b1b=b1b, w2b=w2b.astype(bf), b2b=b2b,
        w3b=w3b, wgb=wgb_host, bgb=bgb, wvb=wvb_host, ident=ident,
    )

    in_maps = []
    for i in range(N_CORES):
        lo = i * n_per_core
        hi = lo + n_per_core
        qp_pad = np.zeros((NQ, 3), dtype=f)
        qp_pad[: hi - lo] = q_pts[lo:hi]
        idx_pad = np.zeros((NQ, K), dtype=np.int64)
        idx_pad[: hi - lo] = neighbor_indices[lo:hi]

        qp_host = np.ascontiguousarray(qp_pad.reshape(T, 128, 3).transpose(1, 0, 2))

        # idx16[p, t, g, s]: gather g of tile t covers logical rows
        # i' = (k - g*KPG)*128 + q, wrapped: w[l, s] = list[s*16 + l]
        idx16 = np.zeros((128, T, GPT, NI // 16), np.int16)
        for t in range(T):
            arr = idx_pad[t * 128 : (t + 1) * 128, :]      # [128 q, K]
            for g in range(GPT):
                lst = arr[:, g * KPG : (g + 1) * KPG].T.reshape(NI)
                idx16[:, t, g, :] = np.tile(
                    lst.reshape(NI // 16, 16).T.astype(np.int16), (8, 1)
                )

        m = dict(shared)
        m.update(qp=qp_host, idx16=idx16)
        in_maps.append(m)
    return in_maps


def kernel(**inputs):
    from concourse.bass_utils import run_bass_kernel_spmd

    T = 20
    inputs = {k: np.asarray(v) for k, v in inputs.items()}
    idx = inputs["neighbor_indices"].astype(np.int64)

    if T not in _NC_CACHE:
        _NC_CACHE[T] = _build_nc(T)
    nc = _NC_CACHE[T]

    in_maps = _host_prep(
        inputs["q_pts"], inputs["s_pts"], inputs["s_feats"], idx,
        inputs["normals"], inputs["W1"], inputs["b1"], inputs["W2"],
        inputs["b2"], inputs["W3"], inputs["b3"], inputs["Wg"],
        inputs["bg"], inputs["Wv"], T,
    )
    res = run_bass_kernel_spmd(nc, in_maps, core_ids=list(range(N_CORES)))

    n_per_core = N // N_CORES
    out = np.empty((N, OUT, 3), dtype=np.float32)
    for i in range(N_CORES):
        o = np.asarray(res.results[i]["out"], dtype=np.float32)
        out[i * n_per_core : (i + 1) * n_per_core] = o.transpose(2, 1, 0)[:n_per_core]
    return out
